# revision 79
# baseline (speedup 1.0000x reference)
"""Masked multi-head attention on 8 Trainium2 NeuronCores.

Sharding: batch x head-group. Core c handles batch c//4 and heads
4*(c%4) .. 4*(c%4)+3 (Wq/Wk/Wv column-sharded, Wo row-sharded). Each core
computes a partial [S, D_MODEL] output = attn_heads @ Wo_slice; the host sums
the 4 partials per batch (the row-parallel reduce) and adds bo + bv @ Wo
(the bv term folds out because softmax rows sum to 1).

All DRAM I/O is bf16 in SBUF-native layouts prepared on the host (one fat
DMA per tensor/block). Device kernel per 512-wide s block j:
  projections for j (psum f32, bf16 operands) -> attention for 4 heads over
  sk-tile pairs (scores into [128,1024] psum pairs, one wide exp per pair,
  causal masking only on diagonal pairs, row sums via a fused ones-column in
  the V stationary) -> output projection. PE issue order software-pipelines
  the projections of block j+1 and the output projection of block j-1 into
  the attention slots of block j so the tensor engine never waits on exp.
"""

import numpy as np

D_MODEL = 1024
N_HEAD = 16
HEAD_DIM = 64
B, S = 2, 2048
GH = 4  # heads per core
GC = GH * HEAD_DIM  # 256 dout columns per core
SBK = 512  # s block (moving free dim)
NSB = S // SBK  # 4 s blocks
NKT = D_MODEL // 128  # 8 din tiles

_CACHE = {}


def _pairs_for(j):
    """sk-tile pairs for attention block j.

    Returns list of (pair_width, items); item = (i, off, w, a0, c0):
    score matmul writes sc[:, off:off+w] from qT[.., c0:c0+w]; av matmul
    accumulates av[0:65, a0:a0+w] from et[:, off:off+w].
    Diagonal tiles are clamped to width >= 256 (sub-256 moving widths run
    at 1/4 PE rate); the extra columns are zeroed via masks.
    """
    out = []
    for g in range(j):
        for half in range(2):
            i0 = 4 * g + 2 * half
            out.append((1024, [(i0, 0, 512, 0, 0), (i0 + 1, 512, 512, 0, 0)]))
    b4 = 4 * j
    out.append((896, [(b4, 0, 512, 0, 0), (b4 + 1, 512, 384, 128, 128)]))
    out.append((512, [(b4 + 2, 0, 256, 256, 256), (b4 + 3, 256, 256, 256, 256)]))
    return out


def _build_nc():
    import concourse.mybir as mybir
    from concourse import bacc, tile

    F32 = mybir.dt.float32
    BF16 = mybir.dt.bfloat16
    EXP = mybir.ActivationFunctionType.Exp

    nc = bacc.Bacc(None, target_bir_lowering=False)

    # DRAM params in SBUF-native layouts (host pre-permutes):
    # x*[p, j*NKT+kt, c] = X^T[kt*128+p, j*512+c]
    xq = nc.declare_dram_parameter("xq", [128, NKT * NSB, SBK], BF16, isOutput=False)
    xk = nc.declare_dram_parameter("xk", [128, NKT * NSB, SBK], BF16, isOutput=False)
    xv = nc.declare_dram_parameter("xv", [128, NKT * NSB, SBK], BF16, isOutput=False)
    wq = nc.declare_dram_parameter("wq", [128, NKT, GC], BF16, isOutput=False)
    wk = nc.declare_dram_parameter("wk", [128, NKT, GC], BF16, isOutput=False)
    wv = nc.declare_dram_parameter("wv", [128, NKT, GC], BF16, isOutput=False)
    wo = nc.declare_dram_parameter("wo", [128, 2, D_MODEL], BF16, isOutput=False)
    bq = nc.declare_dram_parameter("bq", [128, 2], F32, isOutput=False)
    bk = nc.declare_dram_parameter("bk", [128, 2], F32, isOutput=False)
    # rows 0..S-1: blocks 0-2 summed over both pt halves; block 3 is split:
    # rows 1536..2047 = pt0 (heads 0,1) partial, rows S..S+511 = pt1 partial
    y = nc.declare_dram_parameter("y", [S + SBK, D_MODEL], BF16, isOutput=True)

    with tile.TileContext(nc) as tc:
        with (
            tc.tile_pool(name="res", bufs=1) as res,
            tc.tile_pool(name="work", bufs=2) as work,
            tc.tile_pool(name="xin", bufs=2) as xin,
            tc.tile_pool(name="ps", bufs=2, space="PSUM") as ps,
        ):
            # ---- resident weights/biases + j0 activations, interleaved so
            # the first projection matmuls start as soon as possible ----
            bq_sb = res.tile([128, 2], F32, tag="bq")
            bk_sb = res.tile([128, 2], F32, tag="bk")

            wq_sb = res.tile([128, NKT, GC], BF16, tag="wq")
            wk_sb = res.tile([128, NKT, GC], BF16, tag="wk")
            wv_sb = res.tile([128, NKT, GC], BF16, tag="wv")
            wo_sb = res.tile([128, 2, D_MODEL], BF16, tag="wo")

            srcs = {"xq": xq, "xk": xk, "xv": xv}
            x_t = {}

            def new_xt(nm, j):
                t = xin.tile([128, NKT, SBK], BF16, tag=nm, name=f"{nm}_{j}")
                x_t[(nm, j)] = t
                return t

            def load_x_part(nm, j, k0, k1):
                nc.sync.dma_start(
                    x_t[(nm, j)][:, k0:k1], srcs[nm][:, j * NKT + k0 : j * NKT + k1]
                )

            def load_x(nm, j):
                new_xt(nm, j)
                load_x_part(nm, j, 0, 4)
                load_x_part(nm, j, 4, NKT)

            # warmup tile for PE p-state ramp (throwaway matmuls on const data)
            warm = res.tile([128, SBK], BF16, tag="warm")
            nc.gpsimd.memset(warm[:], 0.0)

            def warm_mm(n):
                for _ in range(n):
                    wp = ps.tile([128, SBK], F32, tag="proj", name="warmp")
                    nc.tensor.matmul(
                        wp[:], warm[:, 0:128], warm[:], start=True, stop=True
                    )

            # prologue DMA order = exact consumption order of the j0 q/k
            # projection chunks, then xv0, then block-1 x halves, then wo.
            for nm in ("xq", "xk", "xv"):
                new_xt(nm, 0)
            nc.sync.dma_start(bq_sb[:], bq[:])
            nc.sync.dma_start(bk_sb[:], bk[:])
            nc.sync.dma_start(wq_sb[:, 0:4], wq[:, 0:4])
            load_x_part("xq", 0, 0, 2)
            load_x_part("xq", 0, 2, 4)
            nc.sync.dma_start(wk_sb[:, 0:4], wk[:, 0:4])
            load_x_part("xk", 0, 0, 2)
            load_x_part("xk", 0, 2, 4)
            nc.sync.dma_start(wq_sb[:, 4:8], wq[:, 4:8])
            load_x_part("xq", 0, 4, 6)
            load_x_part("xq", 0, 6, 8)
            nc.sync.dma_start(wk_sb[:, 4:8], wk[:, 4:8])
            load_x_part("xk", 0, 4, 6)
            load_x_part("xk", 0, 6, 8)
            nc.sync.dma_start(wv_sb[:], wv[:])
            load_x_part("xv", 0, 0, 4)
            load_x_part("xv", 0, 4, NKT)

            # Back-to-back dummy matmuls keep PE continuously busy from t~=0
            # so the p-state ramp (3us of continuous execution) completes on
            # throwaway work while the first DMAs land.
            warm_mm(7)

            # ---- causal masks ----
            # maskt [128,128]: keep col >= row; mask3 [128,256] = [0 | maskt]
            maskt = res.tile([128, 128], BF16, tag="maskt")
            nc.gpsimd.memset(maskt[:], 1.0)
            nc.gpsimd.affine_select(
                out=maskt[:],
                in_=maskt[:],
                compare_op=mybir.AluOpType.is_ge,
                fill=0.0,
                base=0,
                pattern=[[1, 128]],
                channel_multiplier=-1,
            )
            mask3 = res.tile([128, 256], BF16, tag="mask3")
            nc.gpsimd.memset(mask3[:], 0.0)
            nc.vector.tensor_copy(mask3[:, 128:256], maskt[:])

            # ---- resident activations ----
            qT_sb = [[res.tile([128, SBK], BF16, tag=f"qT_{pt}_{j}", name=f"qT_{pt}_{j}") for j in range(NSB)] for pt in range(2)]
            kT_sb = [[res.tile([128, SBK], BF16, tag=f"kT_{pt}_{j}", name=f"kT_{pt}_{j}") for j in range(NSB)] for pt in range(2)]
            oT_sb = [[res.tile([128, SBK], BF16, tag=f"oT_{pt}_{j}", name=f"oT_{pt}_{j}") for j in range(NSB)] for pt in range(2)]
            # v_aug[jb]: [128, 4(i in block), GH, 65]; cols 0..63 = v, col 64 = 1
            v_aug = [res.tile([128, 4, GH, HEAD_DIM + 1], BF16, tag=f"vaug_{jb}", name=f"vaug_{jb}") for jb in range(NSB)]
            ones_tmp = res.tile([128, 4, GH], F32, tag="ones_tmp")
            nc.vector.memset(ones_tmp[:], 1.0)
            for jb in range(NSB):
                nc.vector.tensor_copy(v_aug[jb][:, :, :, HEAD_DIM], ones_tmp[:])

            # ---- projection chunk builders (each chunk ~4 matmuls) ----
            def proj_chunks(j):
                chunks = []

                def qk_group(nm, w_sb, b_sb, dst, pt):
                    st = {}

                    def c1():
                        st["p"] = ps.tile([128, SBK], F32, tag="proj", name="projp")
                        for kt in range(4):
                            nc.tensor.matmul(
                                st["p"][:],
                                w_sb[:, kt, pt * 128 : (pt + 1) * 128],
                                x_t[(nm, j)][:, kt],
                                start=(kt == 0),
                                stop=False,
                            )

                    def c2():
                        p = st["p"]
                        for kt in range(4, NKT):
                            nc.tensor.matmul(
                                p[:],
                                w_sb[:, kt, pt * 128 : (pt + 1) * 128],
                                x_t[(nm, j)][:, kt],
                                start=False,
                                stop=(kt == NKT - 1),
                            )
                        nc.vector.tensor_scalar_add(dst[:], p[:], b_sb[:, pt : pt + 1])

                    return [c1, c2]

                def v_group(stv):
                    st = {}

                    def c1():
                        st["p"] = ps.tile([128, SBK], F32, tag="proj", name="projp")
                        for kt in range(4):
                            nc.tensor.matmul(
                                st["p"][:, :GC],
                                x_t[("xv", j)][:, kt, stv * 128 : (stv + 1) * 128],
                                wv_sb[:, kt],
                                start=(kt == 0),
                                stop=False,
                            )

                    def c2():
                        p = st["p"]
                        for kt in range(4, NKT):
                            nc.tensor.matmul(
                                p[:, :GC],
                                x_t[("xv", j)][:, kt, stv * 128 : (stv + 1) * 128],
                                wv_sb[:, kt],
                                start=False,
                                stop=(kt == NKT - 1),
                            )
                        pv3 = p[:, :GC].rearrange("p (h d) -> p h d", h=GH)
                        nc.vector.tensor_copy(v_aug[j][:, stv, :, 0:HEAD_DIM], pv3[:])

                    return [c1, c2]

                qk = []
                for pt in range(2):
                    qk += qk_group("xq", wq_sb, bq_sb, qT_sb[pt][j], pt)
                for pt in range(2):
                    qk += qk_group("xk", wk_sb, bk_sb, kT_sb[pt][j], pt)
                vc = []
                for stv in range(4):
                    vc += v_group(stv)
                return qk, vc

            # ---- output projection chunk builders ----
            def outproj_chunks(j):
                # j=0 fills attention-1 (ACT slack there); later blocks fill
                # exp-bound windows, keep their copies off the ACT engine.
                eng0 = nc.vector.tensor_copy
                chunks = []
                for tt in range(4):
                    st = {}
                    for eb in range(2):
                        def c(tt=tt, eb=eb, st=st):
                            if eb == 0:
                                st["y"] = work.tile([128, D_MODEL], BF16, tag="ysb", bufs=6, name="ysb")
                            yp = ps.tile([128, SBK], F32, tag="proj", name="yp")
                            for pt in range(2):
                                nc.tensor.matmul(
                                    yp[:],
                                    oT_sb[pt][j][:, tt * 128 : (tt + 1) * 128],
                                    wo_sb[:, pt, eb * SBK : (eb + 1) * SBK],
                                    start=(pt == 0),
                                    stop=(pt == 1),
                                )
                            eng = eng0 if eb == 0 else nc.vector.tensor_copy
                            eng(st["y"][:, eb * SBK : (eb + 1) * SBK], yp[:])
                            if eb == 1:
                                t = j * 4 + tt
                                nc.sync.dma_start(
                                    y[t * 128 : (t + 1) * 128, :], st["y"][:]
                                )
                        chunks.append(c)
                return chunks

            # block-3 output projection, split by pt half: pt0 (heads 0,1)
            # runs as fill inside attention-3's second head-pair; pt1 drains
            # at the very end across spare psum tags + 3 copy engines.
            def outproj3_pt(pt):
                chunks = []
                j = NSB - 1
                row0 = j * SBK if pt == 0 else S
                if pt == 0:
                    for tt in range(4):
                        st = {}
                        for eb in range(2):
                            def c(tt=tt, eb=eb, st=st):
                                if eb == 0:
                                    st["y"] = work.tile([128, D_MODEL], BF16, tag="ysb", bufs=6, name="ysb")
                                yp = ps.tile([128, SBK], F32, tag="proj", name="yp")
                                nc.tensor.matmul(
                                    yp[:],
                                    oT_sb[0][j][:, tt * 128 : (tt + 1) * 128],
                                    wo_sb[:, 0, eb * SBK : (eb + 1) * SBK],
                                    start=True,
                                    stop=True,
                                )
                                eng = nc.scalar.copy if eb == 0 else nc.vector.tensor_copy
                                eng(st["y"][:, eb * SBK : (eb + 1) * SBK], yp[:])
                                if eb == 1:
                                    nc.sync.dma_start(
                                        y[row0 + tt * 128 : row0 + (tt + 1) * 128, :],
                                        st["y"][:],
                                    )
                            chunks.append(c)
                    return chunks

                # pt1: one chunk per tt, parallel psum drains
                def mk(tt):
                    def c():
                        ysb = work.tile([128, D_MODEL], BF16, tag="ysb", bufs=6, name="ysb")
                        if tt < 2:
                            yp = ps.tile([128, 2 * SBK], F32, tag="sc", bufs=2, name="yp2")
                            for eb in range(2):
                                nc.tensor.matmul(
                                    yp[:, eb * SBK : (eb + 1) * SBK],
                                    oT_sb[1][j][:, tt * 128 : (tt + 1) * 128],
                                    wo_sb[:, 1, eb * SBK : (eb + 1) * SBK],
                                    start=True,
                                    stop=True,
                                )
                            if tt == 0:
                                nc.scalar.copy(ysb[:], yp[:])
                            else:
                                nc.vector.tensor_copy(ysb[:], yp[:])
                            nc.sync.dma_start(
                                y[S + tt * 128 : S + (tt + 1) * 128, :], ysb[:]
                            )
                        else:
                            tag = "av" if tt == 2 else "proj"
                            for eb in range(2):
                                yp = ps.tile([128, SBK], F32, tag=tag, bufs=2, name="yp1")
                                nc.tensor.matmul(
                                    yp[:],
                                    oT_sb[1][j][:, tt * 128 : (tt + 1) * 128],
                                    wo_sb[:, 1, eb * SBK : (eb + 1) * SBK],
                                    start=True,
                                    stop=True,
                                )
                                eng = nc.scalar.copy if eb == 0 else nc.vector.tensor_copy
                                eng(ysb[:, eb * SBK : (eb + 1) * SBK], yp[:])
                            nc.sync.dma_start(
                                y[S + tt * 128 : S + (tt + 1) * 128, :], ysb[:]
                            )
                    return c

                return [mk(tt) for tt in range(4)]

            # exp'd early-pair tiles of block j, precomputed in block j-1's
            # window (cascading: early blocks have exp-engine slack, the
            # last block's exp stream is saturated)
            PRE_PAIRS = {NSB - 2: (0,), NSB - 1: (0, 1)}
            et3 = {}

            # ---- attention slots for block j ----
            def attention_slots(j):
                slots = []  # (pre, post) callable pairs
                pair_list = _pairs_for(j)
                last_i = 4 * j + 3
                pre_set = PRE_PAIRS.get(j, ())
                # process two live (sc+exp) pairs before the precomputed
                # av-only pairs so the exp engine is fed immediately at each
                # head-pair boundary
                live = [p for p in range(len(pair_list)) if p not in pre_set]
                order = live + list(pre_set)
                first_pair = order[0]
                last_pair = order[-1]
                for hpair in ((0, 1), (2, 3)):
                    av_t = {}
                    for p_idx in order:
                        pw, items = pair_list[p_idx]
                        is_diag1 = p_idx == len(pair_list) - 2
                        is_diag2 = p_idx == len(pair_list) - 1
                        pre_done = p_idx in pre_set
                        scs = {}

                        def pre(hpair=hpair, items=items, scs=scs, pre_done=pre_done):
                            if pre_done:
                                return
                            for h in hpair:
                                pt, po = h // 2, 64 * (h % 2)
                                sc = ps.tile([128, 2 * SBK], F32, tag="sc", bufs=2, name="sc")
                                for (i, off, w, a0, c0) in items:
                                    nc.tensor.matmul(
                                        sc[:, off : off + w],
                                        kT_sb[pt][i // 4][po : po + 64, (i % 4) * 128 : (i % 4 + 1) * 128],
                                        qT_sb[pt][j][po : po + 64, c0 : c0 + w],
                                        start=True,
                                        stop=True,
                                    )
                                scs[h] = sc

                        def post(hpair=hpair, pw=pw, items=items, scs=scs,
                                 is_diag1=is_diag1, is_diag2=is_diag2, av_t=av_t,
                                 pre_done=pre_done, p_idx=p_idx, first_pair=first_pair,
                                 last_pair=last_pair):
                            for h in hpair:
                                pt, po = h // 2, 64 * (h % 2)
                                if pre_done:
                                    et = et3.pop((j, h, p_idx))
                                else:
                                    et = work.tile([128, 2 * SBK], BF16, tag="et", bufs=8, name="et")
                                    nc.scalar.activation(et[:, 0:pw], scs[h][:, 0:pw], EXP, scale=0.125)
                                if is_diag1:
                                    nc.vector.tensor_mul(et[:, 0:128], et[:, 0:128], maskt[:])
                                    nc.vector.tensor_mul(et[:, 512:640], et[:, 512:640], maskt[:])
                                elif is_diag2:
                                    nc.vector.tensor_mul(et[:, 0:128], et[:, 0:128], maskt[:])
                                    nc.vector.tensor_mul(et[:, 256:512], et[:, 256:512], mask3[:])
                                if h not in av_t:
                                    av_t[h] = ps.tile([128, SBK], F32, tag="av", bufs=2, name="av")
                                av = av_t[h]
                                for ii, (i, off, w, a0, c0) in enumerate(items):
                                    nc.tensor.matmul(
                                        av[0:65, a0 : a0 + w],
                                        v_aug[i // 4][:, i % 4, h, :],
                                        et[:, off : off + w],
                                        start=(p_idx == first_pair and ii == 0),
                                        stop=(p_idx == last_pair and ii == len(items) - 1),
                                    )
                                if p_idx == last_pair:
                                    with tc.high_priority(offset=64):
                                        r_inv = work.tile([128, SBK], F32, tag="r_inv", bufs=2, name="r_inv")
                                        rb = work.tile([128, SBK], F32, tag="rb", bufs=2, name="rb")
                                        nc.vector.reciprocal(r_inv[0:1, :], av[64:65, :])
                                        nc.gpsimd.partition_broadcast(rb[:], r_inv[0:1, :])
                                        nc.vector.tensor_mul(
                                            oT_sb[pt][j][po : po + 64, :], av[0:64, :], rb[0:64, :]
                                        )

                        slots.append((pre, post))
                return slots

            # ---- prologue: q/k projections for j=0 with warm-matmul filler
            # (data streams in slower than PE consumes it); v projection is
            # early fill inside attention j0 ----
            qk0, v0 = proj_chunks(0)
            qk_order = (0, 2, 4, 6, 1, 3, 5, 7)  # all c1 chunks, then c2
            warm_after = {6: 3, 3: 3}  # fill DMA-wait gaps before c2 chunks
            for ci, idx in enumerate(qk_order):
                qk0[idx]()
                warm_mm(warm_after.get(idx, 0))

            v_next = v0
            # ---- main loop ----
            for j in range(NSB):
                if j + 1 < NSB:
                    load_x("xq", j + 1)
                    load_x("xk", j + 1)
                    if j == 0:
                        nc.sync.dma_start(wo_sb[:], wo[:])
                    load_x("xv", j + 1)
                slots = attention_slots(j)
                n = len(slots)
                npairs = n // 2
                fills = [[] for _ in range(n)]

                def distribute(items, s0, s1):
                    m = s1 - s0
                    k0 = 0
                    for s in range(m):
                        k1 = (s + 1) * len(items) // m
                        fills[s0 + s] += items[k0:k1]
                        k0 = k1

                # v projection of THIS block fills hpair-1 slots (data is
                # resident); must be fully emitted before the diagonal posts,
                # which sit at position len(live)-1 under the reordering
                distribute(v_next, 0, npairs - len(PRE_PAIRS.get(j, ())))
                if j >= 1 and j < NSB - 1:
                    distribute(outproj_chunks(j - 1), 0, min(n, 4))
                if j + 1 < NSB:
                    qkn, v_next = proj_chunks(j + 1)
                    # x(j+1) lands ~9us into block j; keep its q/k chunks out
                    # of the first slots so PE never queues behind their DMAs
                    distribute(qkn, max(1, n // 3), n)
                else:
                    # attention-3 is exp-bound: feed it the deferred block-2
                    # output projection plus this block's pt0 half
                    distribute(outproj_chunks(2), 6, n - 2)
                    distribute(outproj3_pt(0), npairs + 2, n)
                if True:
                    def pre3(hp, px, jn=0):
                        items0 = _pairs_for(jn)[px][1]

                        def c():
                            for h in hp:
                                pt3, po3 = h // 2, 64 * (h % 2)
                                sc = ps.tile([128, 2 * SBK], F32, tag="sc", bufs=2, name="sc")
                                for (i, off, w, a0, c0) in items0:
                                    nc.tensor.matmul(
                                        sc[:, off : off + w],
                                        kT_sb[pt3][i // 4][po3 : po3 + 64, (i % 4) * 128 : (i % 4 + 1) * 128],
                                        qT_sb[pt3][jn][po3 : po3 + 64, c0 : c0 + w],
                                        start=True,
                                        stop=True,
                                    )
                                et = work.tile([128, 2 * SBK], BF16, tag="etx", bufs=16, name="etx")
                                nc.scalar.activation(et[:, 0:1024], sc[:, 0:1024], EXP, scale=0.125)
                                et3[(jn, h, px)] = et
                        return c

                    nxt = PRE_PAIRS.get(j + 1, ())
                    if len(nxt) == 1:
                        fills[n - 2].append(pre3((0, 1), nxt[0], j + 1))
                        fills[n - 1].append(pre3((2, 3), nxt[0], j + 1))
                    elif len(nxt) == 2:
                        fills[n - 6].append(pre3((0, 1), nxt[0], j + 1))
                        fills[n - 4].append(pre3((2, 3), nxt[0], j + 1))
                        fills[n - 3].append(pre3((0, 1), nxt[1], j + 1))
                        fills[n - 1].append(pre3((2, 3), nxt[1], j + 1))
                    elif len(nxt) == 3:
                        fills[n - 6].append(pre3((0, 1), nxt[0], j + 1))
                        fills[n - 5].append(pre3((2, 3), nxt[0], j + 1))
                        fills[n - 4].append(pre3((0, 1), nxt[1], j + 1))
                        fills[n - 3].append(pre3((2, 3), nxt[1], j + 1))
                        fills[n - 2].append(pre3((0, 1), nxt[2], j + 1))
                        fills[n - 1].append(pre3((2, 3), nxt[2], j + 1))
                # pipelined emission: slot s+1's score matmuls go to the PE
                # stream BEFORE slot s's av matmuls, so the exp engine's next
                # scores are never queued behind avs/fill
                prev_post = None
                for s, (pre, post) in enumerate(slots):
                    pre()
                    if prev_post is not None:
                        prev_post()
                    for f in fills[s]:
                        f()
                    prev_post = post
                prev_post()
            for c in outproj3_pt(1):
                c()
    nc.finalize()
    return nc


def _run_device(Q, K, V, Wq, bq, Wk, bk, Wv, Wo):
    import ml_dtypes
    from concourse.bass_utils import run_bass_kernel_spmd

    BF = ml_dtypes.bfloat16
    if "nc" not in _CACHE:
        _CACHE["nc"] = _build_nc()
    nc = _CACHE["nc"]

    def xlayout(a):
        # [S, D] -> [128, NSB*NKT, SBK]: out[p, j*8+t, c] = a[j*512+c, t*128+p]
        t = a.T.astype(BF)  # [D, S]
        t = t.reshape(NKT, 128, NSB, SBK)
        return np.ascontiguousarray(t.transpose(1, 2, 0, 3).reshape(128, NSB * NKT, SBK))

    def wlayout(w):
        # [D, GC] -> [128, NKT, GC]
        return np.ascontiguousarray(w.astype(BF).reshape(NKT, 128, GC).transpose(1, 0, 2))

    def wolayout(w):
        # [GC, D] -> [128, 2, D]
        return np.ascontiguousarray(w.astype(BF).reshape(2, 128, D_MODEL).transpose(1, 0, 2))

    in_maps = []
    xT = {}
    for b in range(B):
        xT[("q", b)] = xlayout(Q[b])
        xT[("k", b)] = xlayout(K[b])
        xT[("v", b)] = xlayout(V[b])
    for c in range(8):
        b, g = c // 4, c % 4
        cs = slice(g * GC, (g + 1) * GC)
        in_maps.append(
            {
                "xq": xT[("q", b)],
                "xk": xT[("k", b)],
                "xv": xT[("v", b)],
                "wq": wlayout(Wq[:, cs]),
                "wk": wlayout(Wk[:, cs]),
                "wv": wlayout(Wv[:, cs]),
                "wo": wolayout(Wo[cs, :]),
                "bq": np.ascontiguousarray(bq[cs].reshape(2, 128).T),
                "bk": np.ascontiguousarray(bk[cs].reshape(2, 128).T),
            }
        )
    res = run_bass_kernel_spmd(nc, in_maps, core_ids=list(range(8)))
    return res


def kernel(Q, K, V, mask, Wq, bq, Wk, bk, Wv, bv, Wo, bo):
    Q = np.asarray(Q, dtype=np.float32)
    K = np.asarray(K, dtype=np.float32)
    V = np.asarray(V, dtype=np.float32)
    mask = np.asarray(mask)
    Wq, Wk, Wv, Wo = (np.asarray(a, dtype=np.float32) for a in (Wq, Wk, Wv, Wo))
    bq, bk, bv, bo = (np.asarray(a, dtype=np.float32) for a in (bq, bk, bv, bo))

    causal = bool(
        np.array_equal(mask[0], np.tril(np.ones((S, S), dtype=mask.dtype)))
    )
    if not causal:
        return _numpy_fallback(Q, K, V, mask, Wq, bq, Wk, bk, Wv, bv, Wo, bo)

    res = _run_device(Q, K, V, Wq, bq, Wk, bk, Wv, Wo)
    bo_eff = bo + bv @ Wo
    out = np.empty((B, S, D_MODEL), dtype=np.float32)
    for b in range(B):
        acc = res.results[4 * b]["y"].astype(np.float32)
        for g in range(1, 4):
            acc = acc + res.results[4 * b + g]["y"].astype(np.float32)
        # block-3 rows are split into pt0 (rows 1536:2048) + pt1 (rows 2048:)
        acc[S - SBK : S] += acc[S : S + SBK]
        out[b] = acc[:S] + bo_eff
    return out


def _numpy_fallback(Q, K, V, mask, Wq, bq, Wk, bk, Wv, bv, Wo, bo):
    out = np.empty((B, S, D_MODEL), dtype=np.float32)
    for b in range(B):
        q = (Q[b] @ Wq + bq).reshape(S, N_HEAD, HEAD_DIM).transpose(1, 0, 2)
        k = (K[b] @ Wk + bk).reshape(S, N_HEAD, HEAD_DIM).transpose(1, 0, 2)
        v = (V[b] @ Wv + bv).reshape(S, N_HEAD, HEAD_DIM).transpose(1, 0, 2)
        mb = mask[b] if mask.shape[0] > 1 else mask[0]
        o = np.empty((N_HEAD, S, HEAD_DIM), dtype=np.float32)
        for hh in range(N_HEAD):
            s = (q[hh] @ k[hh].T) / np.sqrt(np.float32(HEAD_DIM))
            s = np.where(mb == 0, -np.inf, s)
            s = s - s.max(-1, keepdims=True)
            e = np.exp(s)
            p = e / e.sum(-1, keepdims=True)
            o[hh] = p @ v[hh]
        out[b] = o.transpose(1, 0, 2).reshape(S, D_MODEL) @ Wo + bo
    return out
